# revision 22
# baseline (speedup 1.0000x reference)
"""Braid causal self-attention (sigmoid attention + RoPE + QK RMS-norm) on 8
Trainium2 NeuronCores, tensor-parallel over heads (2 heads per core).

Contract: kernel(**inputs) takes the FULL unsharded inputs (numpy) and returns
the FULL output [1, 4096, 1024] float32.

Sharding (host side, per core c):
  - wq/wk/wv rows [128c, 128c+128) == heads 2c, 2c+1, concatenated and
    transposed into one fused [1024, 384] bf16 "wqkv" operand.
  - wproj cols [128c, 128c+128), transposed to [128, 1024] bf16, pre-scaled
    by 1/(sqrt(T)+1e-6).
  - x is pre-transposed on host to xT [1024, 4096] bf16 (so the device needs
    no PE transposes for the QKV contractions).
  - cos/sin are duplicated x4 on host to [4096, 128] bf16 so RoPE group views
    line up without broadcast APs.
  - Each core computes a full-shape partial output y_c @ wproj_c.T; host sums
    the 8 partials.

Device pipeline per core (all bf16 matmuls, fp32 accumulation), fully fused
emission so PE / ACT / DVE / Pool / DMA overlap:
  p1 (per 128-row tile): DMA xT tile, 8 fused-QKV matmuls (N=384) into one
     PSUM bank, RoPE + per-head RMS-norm on DVE, q-hat/k-hat transposed into
     resident qT/kT via DMA XBAR transpose (no PE), v kept natural.
  p2 (per 512-col q chunk, causal tiles only): scoresT = kT.T @ qT (K=64),
     sigmoid on ACT straight out of PSUM, memset/tri-mask on Pool,
     yT += v.T @ attnT accumulated in one PSUM bank for both heads.
  p3 (per 128-row tile): out = yT.T @ wprojT, PSUM -> SBUF -> DRAM f32.
Emission order: p1(0..3), then per qc: p2(qc), p1(next 4 tiles), p3(qc) --
keeps the tensor engine continuously busy (stays at full 2.4 GHz p-state).
"""

import sys

sys.path.insert(0, "/opt/trn_rl_repo")

import numpy as np
import ml_dtypes

import concourse.bass as bass
import concourse.mybir as mybir
from concourse import bacc
from concourse.tile import TileContext
from concourse.bass_utils import run_bass_kernel_spmd
from concourse.masks import make_upper_triangular

T = 4096
C = 1024
N_CORES = 8
D = 64  # head dim
H_PER_CORE = 2  # heads per core
DSH = D * H_PER_CORE  # 128, per-core qkv width
TT = T // 128  # 32 row tiles
C8 = C // 128  # 8 contraction chunks
QC = T // 512  # 8 q chunks
EPS = 1e-6

F32 = mybir.dt.float32
BF16 = mybir.dt.bfloat16
BF_NP = ml_dtypes.bfloat16

_COMPILED = None


def _build():
    nc = bacc.Bacc("TRN2", target_bir_lowering=False, debug=False,
                   num_devices=N_CORES, num_swdge_queues=4)

    xT_d = nc.dram_tensor("xT", [T * 128 // 128, C], BF16, kind="ExternalInput")  # [TT*128, 1024] tiled
    cos4_d = nc.dram_tensor("cos4", [T, 256], BF16, kind="ExternalInput")
    sin4_d = nc.dram_tensor("sin4", [T, 256], BF16, kind="ExternalInput")
    wqkv_d = nc.dram_tensor("wqkv", [C, 384], BF16, kind="ExternalInput")
    wpT_d = nc.dram_tensor("wpT", [DSH, C], BF16, kind="ExternalInput")
    out_d = nc.dram_tensor("out", [T, C], BF16, kind="ExternalOutput")

    mul = mybir.AluOpType.mult
    sig = mybir.ActivationFunctionType.Sigmoid
    sqrtf = mybir.ActivationFunctionType.Sqrt

    with TileContext(nc) as tc:
        with (
            tc.tile_pool(name="const", bufs=1) as constp,
            tc.tile_pool(name="resident", bufs=1) as resp,
            tc.tile_pool(name="xp", bufs=16) as xp,
            tc.tile_pool(name="rw", bufs=2) as rw,
            tc.tile_pool(name="qkvps", bufs=2, space="PSUM") as qkvps,
            tc.tile_pool(name="sps", bufs=3, space="PSUM") as sps,
            tc.tile_pool(name="yp", bufs=2, space="PSUM") as yp,
            tc.tile_pool(name="op", bufs=1, space="PSUM") as op,
            tc.tile_pool(name="atp", bufs=3) as atp,
            tc.tile_pool(name="osb", bufs=2) as osb,
        ):
            # fused qkv weights: wqkv_b[p, c8, j]: contraction chunk c8 rows
            # at partitions p; j = (q 0:128 | k 128:256 | v 256:384).
            # Split into halves so the first QKV matmuls start early.
            wqkv_b = constp.tile([128, C8, 384], BF16)
            wqkv_r = wqkv_d.rearrange("(n p) j -> p n j", p=128)
            nc.gpsimd.dma_start(out=wqkv_b[:, 0:4, :], in_=wqkv_r[:, 0:4, :])
            nc.gpsimd.dma_start(out=wqkv_b[:, 4:8, :], in_=wqkv_r[:, 4:8, :])
            wp_b = constp.tile([128, C], BF16)

            # cos/sin duplicated x4 host-side: [p, tile, 128]; first-half
            # tiles land first so p1(0) RoPE is not blocked.
            cosb = constp.tile([128, TT, 256], BF16)
            sinb = constp.tile([128, TT, 256], BF16)
            cos_r = cos4_d.rearrange("(n p) d -> p n d", p=128)
            sin_r = sin4_d.rearrange("(n p) d -> p n d", p=128)
            nc.gpsimd.dma_start(out=cosb[:, 0:4, :], in_=cos_r[:, 0:4, :])
            nc.gpsimd.dma_start(out=sinb[:, 0:4, :], in_=sin_r[:, 0:4, :])
            nc.gpsimd.dma_start(out=cosb[:, 4:TT, :], in_=cos_r[:, 4:TT, :])
            nc.gpsimd.dma_start(out=sinb[:, 4:TT, :], in_=sin_r[:, 4:TT, :])

            # tri[k, q] = 1 where k <= q (valid causal region of scoresT)
            tri = constp.tile([128, 128], BF16)
            make_upper_triangular(nc, tri, val=1.0, diag=True)

            # resident activations, split per 512-token batch so reads
            # depend only on their batch (dep tracking is per tile object)
            qTs = [resp.tile([128, 512], BF16, tag=f"qT{b}", name=f"qT{b}")
                   for b in range(QC)]  # [d(2 heads), t]
            kTs = [resp.tile([128, 512], BF16, tag=f"kT{b}", name=f"kT{b}")
                   for b in range(QC)]
            vNs = [resp.tile([128, 4, DSH], BF16, tag=f"vN{b}", name=f"vN{b}")
                   for b in range(QC)]  # [t_in_tile, tile_in_batch, d]
            yTs = [resp.tile([128, 512], BF16, tag=f"yT{b}", name=f"yT{b}")
                   for b in range(QC)]  # [d(2 heads), t]

            xts = {}

            def p1_load(ti):
                # host-tiled layout: row (128*ti + p) holds all 8 c-chunks
                # for tile ti, partition p -- one contiguous 2KB line per
                # partition, so SP HWDGE issuance is cheap
                xt = xp.tile([128, C8, 128], BF16, tag="xt")
                nc.sync.dma_start(
                    out=xt[:, :, :],
                    in_=xT_d[ti * 128:(ti + 1) * 128, :]
                    .rearrange("p (n j) -> p n j", n=C8))
                xts[ti] = xt

            def p1_compute(ti):
                r0 = ti * 128
                xt = xts.pop(ti)

                qkv = qkvps.tile([128, 384], F32, tag="qkv")
                for c8 in range(C8):
                    nc.tensor.matmul(qkv[:, :], xt[:, c8, :],
                                     wqkv_b[:, c8, :],
                                     start=(c8 == 0), stop=(c8 == C8 - 1))

                # v: straight cast to resident natural layout
                nc.vector.tensor_copy(vNs[ti // 4][:, ti % 4, :],
                                      qkv[:, 256:384])

                # q|k to bf16 sbuf for cheap (2x-mode) DVE RoPE
                qk = rw.tile([128, 256], BF16, tag="qk")
                nc.vector.tensor_copy(qk[:, :], qkv[:, 0:256])

                # 4 groups g = (q_h0, q_h1, k_h0, k_h1); per group x1 = cols
                # [64g, 64g+32), x2 = [64g+32, 64g+64).  Elementwise p1 work
                # alternates DVE / GpSimd by tile parity to halve the per-
                # batch chain latency (the PSUM copies above stay on DVE).
                ew = nc.vector
                rot = rw.tile([128, 256], BF16, tag="rot")
                qkc = rw.tile([128, 256], BF16, tag="qkc")
                qks = rw.tile([128, 256], BF16, tag="qks")
                # full-width products against x8-duplicated cos/sin tables
                ew.tensor_tensor(qkc[:, :], qk[:, :], cosb[:, ti, :], mul)
                ew.tensor_tensor(qks[:, :], qk[:, :], sinb[:, ti, :], mul)
                gv = "p (g t x) -> p g t x"
                x1c = qkc[:, :].rearrange(gv, g=4, t=2)[:, :, 0:1, :]
                x2c = qkc[:, :].rearrange(gv, g=4, t=2)[:, :, 1:2, :]
                x1s = qks[:, :].rearrange(gv, g=4, t=2)[:, :, 0:1, :]
                x2s = qks[:, :].rearrange(gv, g=4, t=2)[:, :, 1:2, :]
                r1 = rot[:, :].rearrange(gv, g=4, t=2)[:, :, 0:1, :]
                r2 = rot[:, :].rearrange(gv, g=4, t=2)[:, :, 1:2, :]
                # rot1 = x1*c + x2*s ; rot2 = x2*c - x1*s
                ew.tensor_tensor(r1, x1c, x2s, mybir.AluOpType.add)
                ew.tensor_tensor(r2, x2c, x1s, mybir.AluOpType.subtract)

                # per-group sum of squares (rotation preserves the norm)
                ssq = rw.tile([128, 4], F32, tag="ssq")
                sqs = rw.tile([128, 256], BF16, tag="sqs")
                ew.tensor_tensor(sqs[:, :], rot[:, :], rot[:, :], mul)
                nc.vector.reduce_sum(
                    ssq[:, :],
                    sqs[:, :].rearrange("p (g x) -> p g x", g=4),
                    axis=mybir.AxisListType.X)
                # inv = sqrt(1/ssq) via two DVE Newton steps: the ACT
                # engine stays sigmoid-only (one table load for the whole
                # kernel, and p1 never blocks the ACT queue head).
                rec = rw.tile([128, 4], F32, tag="rec")
                nc.vector.reciprocal(rec[:, :], ssq[:, :])
                y0 = rw.tile([128, 4], F32, tag="y0")
                nc.vector.tensor_scalar(y0[:, :], rec[:, :], 2.3667, 0.0959,
                                        mul, mybir.AluOpType.add)
                r1n = rw.tile([128, 4], F32, tag="r1n")
                nc.vector.reciprocal(r1n[:, :], y0[:, :])
                t1n = rw.tile([128, 4], F32, tag="t1n")
                nc.vector.tensor_tensor(t1n[:, :], rec[:, :], r1n[:, :], mul)
                y1u = rw.tile([128, 4], F32, tag="y1u")  # = 2*y1
                nc.vector.tensor_tensor(y1u[:, :], y0[:, :], t1n[:, :],
                                        mybir.AluOpType.add)
                r2n = rw.tile([128, 4], F32, tag="r2n")  # = 1/(2*y1)
                nc.vector.reciprocal(r2n[:, :], y1u[:, :])
                t2n = rw.tile([128, 4], F32, tag="t2n")  # = a/(2*y1)
                nc.vector.tensor_tensor(t2n[:, :], rec[:, :], r2n[:, :], mul)
                inv = rw.tile([128, 4], F32, tag="inv")
                # y2 = 0.25*y1u + a/(2*y1); q groups get sqrt(rec) (the 1/8
                # fold), k groups get 8*sqrt(rec) == rsqrt(ssq/64)
                nc.vector.tensor_scalar(inv[:, :], y1u[:, :], 0.25, None,
                                        mul)
                nc.vector.tensor_tensor(inv[:, :], inv[:, :], t2n[:, :],
                                        mybir.AluOpType.add)
                nc.vector.tensor_scalar_mul(inv[:, 2:4], inv[:, 2:4], 8.0)
                _p1_norm_tail(ti, rot, inv)

            def _p1_norm_tail(ti, rot, inv):
                nrm = rw.tile([128, 256], BF16, tag="nrm")
                for g in range(4):
                    nc.vector.tensor_scalar_mul(
                        nrm[:, g * 64:(g + 1) * 64],
                        rot[:, g * 64:(g + 1) * 64],
                        inv[:, g:g + 1])
                # normalized q|k into resident [d, t] via DMA XBAR transpose
                b, c0 = ti // 4, (ti % 4) * 128
                nc.sync.dma_start(out=qTs[b][:, c0:c0 + 128],
                                  in_=nrm[:, 0:128], transpose=True)
                nc.sync.dma_start(out=kTs[b][:, c0:c0 + 128],
                                  in_=nrm[:, 128:256], transpose=True)

            def p2(qc, fillers=(), W=512):
                q0 = qc * 512
                visits = []
                for part in range(512 // W):
                    tqa = part * W
                    nkt_p = (q0 + tqa + W) // 128
                    for h in range(H_PER_CORE):
                        visits.append((part, h, nkt_p))
                m_total = sum(v[2] for v in visits)
                emitted = 0
                m = 0
                ypt = yp.tile([128, 512], F32, tag="y")
                for part, h, nkt_p in visits:
                    tqa = part * W
                    hs = h * 64
                    pend = None  # software-pipeline: av trails s by one kt

                    def av(pkt, pat, is_last):
                        # masked cols [0:jcr) of diagonal tiles skipped
                        pjcr = max(pkt * 128 - (q0 + tqa), 0)
                        nc.tensor.matmul(
                            ypt[hs:hs + 64, tqa + pjcr:tqa + W],
                            vNs[pkt // 4][:, pkt % 4, hs:hs + 64],
                            pat[:, pjcr:W],
                            start=(pkt == 0), stop=is_last,
                            skip_group_check=True)

                    for kt in range(nkt_p):
                        while emitted * m_total < m * len(fillers):
                            fillers[emitted]()
                            emitted += 1
                        m += 1
                        kb, kc = kt // 4, (kt % 4) * 128
                        jcr = max(kt * 128 - (q0 + tqa), 0)
                        s_ps = sps.tile([128, W], F32, tag="s")
                        nc.tensor.matmul(
                            s_ps[:, jcr:W],
                            kTs[kb][hs:hs + 64, kc:kc + 128],
                            qTs[qc][hs:hs + 64, tqa + jcr:tqa + W],
                            start=True, stop=True)
                        at = atp.tile([128, W], BF16, tag="at")
                        nc.scalar.activation(
                            at[:, jcr:W], s_ps[:, jcr:W], sig)
                        if kt * 128 >= q0 + tqa:
                            # diagonal 128-block: triangular mask
                            nc.gpsimd.tensor_tensor(
                                at[:, jcr:jcr + 128], at[:, jcr:jcr + 128],
                                tri[:, :], mul)
                        if pend is not None:
                            av(pend[0], pend[1], False)
                        pend = (kt, at)
                    av(pend[0], pend[1], True)
                while emitted < len(fillers):
                    fillers[emitted]()
                    emitted += 1
                nc.vector.tensor_copy(yTs[qc][:, :], ypt[:, :])

            def p3_half(r, half):
                rb, rc = r // 4, (r % 4) * 128
                r0 = r * 128
                o_ps = op.tile([128, 512], F32, tag="o")
                nc.tensor.matmul(
                    o_ps[:, :], yTs[rb][:, rc:rc + 128],
                    wp_b[:, half * 512:(half + 1) * 512],
                    start=True, stop=True)
                ob = osb.tile([128, 512], BF16, tag="ob")
                if r >= 4 * (QC - 1):
                    nc.scalar.activation(ob[:, :], o_ps[:, :],
                                         mybir.ActivationFunctionType.Copy)
                else:
                    nc.vector.tensor_copy(ob[:, :], o_ps[:, :])
                nc.sync.dma_start(
                    out=out_d[r0:r0 + 128, half * 512:(half + 1) * 512],
                    in_=ob[:, :])

            for ti in range(12):
                p1_load(ti)
            for ti in range(8):
                p1_compute(ti)
            for qc in range(QC):
                # fillers injected INSIDE p2's kt loop: spreads each p1
                # tile's DVE chain / Pool work thin so queue heads never
                # clog.  Loads first (gpsimd), then computes (batch qc+2,
                # 2-chunk slack) alternating with the previous chunk's
                # projection halves.
                fillers = []
                if qc < QC - 3:
                    fillers += [(lambda t=t: p1_load(t))
                                for t in range(4 * qc + 12, 4 * qc + 16)]
                work = []
                if qc < QC - 2:
                    work += [(lambda t=t: p1_compute(t))
                             for t in range(4 * qc + 8, 4 * qc + 12)]
                if qc >= 1:
                    p3w = [(lambda r=r, h2=h2: p3_half(r, h2))
                           for r in range(4 * (qc - 1), 4 * qc)
                           for h2 in range(2)]
                else:
                    p3w = []
                ci, pi = 0, 0
                while ci < len(work) or pi < len(p3w):
                    if ci < len(work):
                        fillers.append(work[ci]); ci += 1
                    for _ in range(2):
                        if pi < len(p3w):
                            fillers.append(p3w[pi]); pi += 1
                p2(qc, fillers, W=(128 if qc == 0 else
                                     256 if qc == 1 else 512))
                if qc == 0:
                    nc.sync.dma_start(out=wp_b[:, :], in_=wpT_d[:, :])
            for r in range(4 * (QC - 1), 4 * QC):
                for half in range(2):
                    p3_half(r, half)

    nc.compile()
    return nc


def _in_maps(x, cos, sin, wq, wk, wv, wproj):
    x2d = np.asarray(x, dtype=np.float32).reshape(T, C)
    # tiled transpose: row (128*ti + p) = [x2d[128*ti + j, 128*n + p]
    # for n in 0..7 for j in 0..127] -- contiguous per-partition lines
    xT_bf = np.ascontiguousarray(
        x2d.reshape(TT, 128, C8, 128).transpose(0, 3, 2, 1)
        .reshape(T, C)).astype(BF_NP)
    cos4 = np.ascontiguousarray(
        np.tile(np.asarray(cos, dtype=np.float32), (1, 8))).astype(BF_NP)
    sin4 = np.ascontiguousarray(
        np.tile(np.asarray(sin, dtype=np.float32), (1, 8))).astype(BF_NP)

    in_maps = []
    for c in range(N_CORES):
        sl = slice(c * DSH, (c + 1) * DSH)
        wcat = np.concatenate(
            [wq[sl, :].T, wk[sl, :].T, wv[sl, :].T], axis=1)
        in_maps.append({
            "xT": xT_bf,
            "cos4": cos4,
            "sin4": sin4,
            "wqkv": np.ascontiguousarray(wcat).astype(BF_NP),
            # fold y/(sqrt(T)+1e-6) into the projection weights
            "wpT": np.ascontiguousarray(
                wproj[:, sl].T * np.float32(1.0 / (64.0 + 1e-6))
            ).astype(BF_NP),
        })
    return in_maps


def kernel(x, cos, sin, wq, wk, wv, wproj):
    global _COMPILED
    if _COMPILED is None:
        _COMPILED = _build()
    nc = _COMPILED

    in_maps = _in_maps(x, cos, sin, wq, wk, wv, wproj)
    res = run_bass_kernel_spmd(nc, in_maps, list(range(N_CORES)))
    acc = np.zeros((T, C), dtype=np.float64)
    for c in range(N_CORES):
        acc += np.asarray(res.results[c]["out"], dtype=np.float32)
    return acc.astype(np.float32).reshape(1, T, C)


# revision 25
# speedup vs baseline: 1.0006x; 1.0006x over previous
"""Braid causal self-attention (sigmoid attention + RoPE + QK RMS-norm) on 8
Trainium2 NeuronCores, tensor-parallel over heads (2 heads per core).

Contract: kernel(**inputs) takes the FULL unsharded inputs (numpy) and returns
the FULL output [1, 4096, 1024] float32.

Sharding (host side, per core c):
  - wq/wk/wv rows [128c, 128c+128) == heads 2c, 2c+1, concatenated and
    transposed into one fused [1024, 384] bf16 "wqkv" operand.
  - wproj cols [128c, 128c+128), transposed to [128, 1024] bf16, pre-scaled
    by 1/(sqrt(T)+1e-6).
  - x is pre-transposed on host to xT [1024, 4096] bf16 (so the device needs
    no PE transposes for the QKV contractions).
  - cos/sin are duplicated x4 on host to [4096, 128] bf16 so RoPE group views
    line up without broadcast APs.
  - Each core computes a full-shape partial output y_c @ wproj_c.T; host sums
    the 8 partials.

Device pipeline per core (all bf16 matmuls, fp32 accumulation), fully fused
emission so PE / ACT / DVE / Pool / DMA overlap:
  p1 (per 128-row tile): DMA xT tile, 8 fused-QKV matmuls (N=384) into one
     PSUM bank, RoPE + per-head RMS-norm on DVE, q-hat/k-hat transposed into
     resident qT/kT via DMA XBAR transpose (no PE), v kept natural.
  p2 (per 512-col q chunk, causal tiles only): scoresT = kT.T @ qT (K=64),
     sigmoid on ACT straight out of PSUM, memset/tri-mask on Pool,
     yT += v.T @ attnT accumulated in one PSUM bank for both heads.
  p3 (per 128-row tile): out = yT.T @ wprojT, PSUM -> SBUF -> DRAM f32.
Emission order: p1(0..3), then per qc: p2(qc), p1(next 4 tiles), p3(qc) --
keeps the tensor engine continuously busy (stays at full 2.4 GHz p-state).
"""

import sys

sys.path.insert(0, "/opt/trn_rl_repo")

import numpy as np
import ml_dtypes

import concourse.bass as bass
import concourse.mybir as mybir
from concourse import bacc
from concourse.tile import TileContext
from concourse.bass_utils import run_bass_kernel_spmd
from concourse.masks import make_upper_triangular

T = 4096
C = 1024
N_CORES = 8
D = 64  # head dim
H_PER_CORE = 2  # heads per core
DSH = D * H_PER_CORE  # 128, per-core qkv width
TT = T // 128  # 32 row tiles
C8 = C // 128  # 8 contraction chunks
QC = T // 512  # 8 q chunks
EPS = 1e-6

F32 = mybir.dt.float32
BF16 = mybir.dt.bfloat16
BF_NP = ml_dtypes.bfloat16

_COMPILED = None


def _build():
    nc = bacc.Bacc("TRN2", target_bir_lowering=False, debug=False,
                   num_devices=N_CORES, num_swdge_queues=4)

    xT_d = nc.dram_tensor("xT", [T * 128 // 128, C], BF16, kind="ExternalInput")  # [TT*128, 1024] tiled
    cos4_d = nc.dram_tensor("cos4", [T, 256], BF16, kind="ExternalInput")
    sin4_d = nc.dram_tensor("sin4", [T, 256], BF16, kind="ExternalInput")
    wqkv_d = nc.dram_tensor("wqkv", [C, 384], BF16, kind="ExternalInput")
    wpT_d = nc.dram_tensor("wpT", [DSH, C], BF16, kind="ExternalInput")
    out_d = nc.dram_tensor("out", [T, C], BF16, kind="ExternalOutput")

    mul = mybir.AluOpType.mult
    sig = mybir.ActivationFunctionType.Sigmoid
    sqrtf = mybir.ActivationFunctionType.Sqrt

    with TileContext(nc) as tc:
        with (
            tc.tile_pool(name="const", bufs=1) as constp,
            tc.tile_pool(name="resident", bufs=1) as resp,
            tc.tile_pool(name="xp", bufs=16) as xp,
            tc.tile_pool(name="rw", bufs=2) as rw,
            tc.tile_pool(name="qkvps", bufs=2, space="PSUM") as qkvps,
            tc.tile_pool(name="sps", bufs=3, space="PSUM") as sps,
            tc.tile_pool(name="yp", bufs=2, space="PSUM") as yp,
            tc.tile_pool(name="op", bufs=1, space="PSUM") as op,
            tc.tile_pool(name="atp", bufs=3) as atp,
            tc.tile_pool(name="osb", bufs=2) as osb,
        ):
            # fused qkv weights: wqkv_b[p, c8, j]: contraction chunk c8 rows
            # at partitions p; j = (q 0:128 | k 128:256 | v 256:384).
            # Split into halves so the first QKV matmuls start early.
            wqkv_b = constp.tile([128, C8, 384], BF16)
            wqkv_r = wqkv_d.rearrange("(n p) j -> p n j", p=128)
            nc.gpsimd.dma_start(out=wqkv_b[:, 0:4, :], in_=wqkv_r[:, 0:4, :])
            nc.gpsimd.dma_start(out=wqkv_b[:, 4:8, :], in_=wqkv_r[:, 4:8, :])
            wp_b = constp.tile([128, C], BF16)

            # cos/sin duplicated x4 host-side: [p, tile, 128]; first-half
            # tiles land first so p1(0) RoPE is not blocked.
            cosb = constp.tile([128, TT, 256], BF16)
            sinb = constp.tile([128, TT, 256], BF16)
            cos_r = cos4_d.rearrange("(n p) d -> p n d", p=128)
            sin_r = sin4_d.rearrange("(n p) d -> p n d", p=128)
            nc.gpsimd.dma_start(out=cosb[:, 0:4, :], in_=cos_r[:, 0:4, :])
            nc.gpsimd.dma_start(out=sinb[:, 0:4, :], in_=sin_r[:, 0:4, :])
            nc.gpsimd.dma_start(out=cosb[:, 4:TT, :], in_=cos_r[:, 4:TT, :])
            nc.gpsimd.dma_start(out=sinb[:, 4:TT, :], in_=sin_r[:, 4:TT, :])

            # tri[k, q] = 1 where k <= q (valid causal region of scoresT)
            tri = constp.tile([128, 128], BF16)
            make_upper_triangular(nc, tri, val=1.0, diag=True)

            # resident activations, split per 512-token batch so reads
            # depend only on their batch (dep tracking is per tile object)
            qTs = [resp.tile([128, 512], BF16, tag=f"qT{b}", name=f"qT{b}")
                   for b in range(QC)]  # [d(2 heads), t]
            kTs = [resp.tile([128, 512], BF16, tag=f"kT{b}", name=f"kT{b}")
                   for b in range(QC)]
            vNs = [resp.tile([128, 4, DSH], BF16, tag=f"vN{b}", name=f"vN{b}")
                   for b in range(QC)]  # [t_in_tile, tile_in_batch, d]
            yTs = [resp.tile([128, 512], BF16, tag=f"yT{b}", name=f"yT{b}")
                   for b in range(QC)]  # [d(2 heads), t]
            ssqB = [resp.tile([128, 16], F32, tag=f"sq{b}", name=f"sq{b}")
                    for b in range(QC)]  # per-batch sum-of-squares
            invB = [resp.tile([128, 16], F32, tag=f"iv{b}", name=f"iv{b}")
                    for b in range(QC)]  # per-batch rsqrt factors

            xts = {}
            rots = {}

            def p1_load(ti):
                # host-tiled layout: row (128*ti + p) holds all 8 c-chunks
                # for tile ti, partition p -- one contiguous 2KB line per
                # partition, so SP HWDGE issuance is cheap
                xt = xp.tile([128, C8, 128], BF16, tag="xt")
                nc.sync.dma_start(
                    out=xt[:, :, :],
                    in_=xT_d[ti * 128:(ti + 1) * 128, :]
                    .rearrange("p (n j) -> p n j", n=C8))
                xts[ti] = xt

            def p1_compute(ti):
                r0 = ti * 128
                xt = xts.pop(ti)

                qkv = qkvps.tile([128, 384], F32, tag="qkv")
                for c8 in range(C8):
                    nc.tensor.matmul(qkv[:, :], xt[:, c8, :],
                                     wqkv_b[:, c8, :],
                                     start=(c8 == 0), stop=(c8 == C8 - 1))

                # v: straight cast to resident natural layout
                nc.vector.tensor_copy(vNs[ti // 4][:, ti % 4, :],
                                      qkv[:, 256:384])

                # q|k to bf16 sbuf for cheap (2x-mode) DVE RoPE
                qk = rw.tile([128, 256], BF16, tag="qk")
                nc.vector.tensor_copy(qk[:, :], qkv[:, 0:256])

                # 4 groups g = (q_h0, q_h1, k_h0, k_h1); per group x1 = cols
                # [64g, 64g+32), x2 = [64g+32, 64g+64).  Elementwise p1 work
                # alternates DVE / GpSimd by tile parity to halve the per-
                # batch chain latency (the PSUM copies above stay on DVE).
                ew = nc.vector
                rot = rw.tile([128, 256], BF16, tag="rot", bufs=10)
                qkc = rw.tile([128, 256], BF16, tag="qkc")
                qks = rw.tile([128, 256], BF16, tag="qks")
                # full-width products against x8-duplicated cos/sin tables
                ew.tensor_tensor(qkc[:, :], qk[:, :], cosb[:, ti, :], mul)
                ew.tensor_tensor(qks[:, :], qk[:, :], sinb[:, ti, :], mul)
                gv = "p (g t x) -> p g t x"
                x1c = qkc[:, :].rearrange(gv, g=4, t=2)[:, :, 0:1, :]
                x2c = qkc[:, :].rearrange(gv, g=4, t=2)[:, :, 1:2, :]
                x1s = qks[:, :].rearrange(gv, g=4, t=2)[:, :, 0:1, :]
                x2s = qks[:, :].rearrange(gv, g=4, t=2)[:, :, 1:2, :]
                r1 = rot[:, :].rearrange(gv, g=4, t=2)[:, :, 0:1, :]
                r2 = rot[:, :].rearrange(gv, g=4, t=2)[:, :, 1:2, :]
                # rot1 = x1*c + x2*s ; rot2 = x2*c - x1*s
                ew.tensor_tensor(r1, x1c, x2s, mybir.AluOpType.add)
                ew.tensor_tensor(r2, x2c, x1s, mybir.AluOpType.subtract)

                # per-group sum of squares (rotation preserves the norm),
                # written into the per-batch ssq tile; the Newton rsqrt runs
                # once per 4-tile batch (see p1_newton)
                b, s4 = ti // 4, (ti % 4) * 4
                sqs = rw.tile([128, 256], BF16, tag="sqs")
                ew.tensor_tensor(sqs[:, :], rot[:, :], rot[:, :], mul)
                nc.vector.reduce_sum(
                    ssqB[b][:, s4:s4 + 4],
                    sqs[:, :].rearrange("p (g x) -> p g x", g=4),
                    axis=mybir.AxisListType.X)
                rots[ti] = rot

            def p1_newton(b):
                # inv = sqrt(1/ssq), one DVE Newton step from a chord seed,
                # batched over 4 tiles (16 groups).  ACT stays sigmoid-only.
                rec = rw.tile([128, 16], F32, tag="rec")
                nc.vector.reciprocal(rec[:, :], ssqB[b][:, :])
                y0 = rw.tile([128, 16], F32, tag="y0")
                nc.vector.tensor_scalar(y0[:, :], rec[:, :], 2.3667, 0.0959,
                                        mul, mybir.AluOpType.add)
                r1n = rw.tile([128, 16], F32, tag="r1n")
                nc.vector.reciprocal(r1n[:, :], y0[:, :])
                t1n = rw.tile([128, 16], F32, tag="t1n")
                nc.vector.tensor_tensor(t1n[:, :], rec[:, :], r1n[:, :], mul)
                # inv = 0.5*(y0 + a/y0); k groups (cols 2:4 mod 4) get x8
                inv = invB[b]
                nc.vector.tensor_tensor(inv[:, :], y0[:, :], t1n[:, :],
                                        mybir.AluOpType.add)
                iv4 = inv[:, :].rearrange("p (t g) -> p t g", t=4)
                nc.vector.tensor_scalar_mul(iv4[:, :, 0:2], iv4[:, :, 0:2],
                                            0.5)
                nc.vector.tensor_scalar_mul(iv4[:, :, 2:4], iv4[:, :, 2:4],
                                            4.0)

            def p1_norm(ti):
                _p1_norm_tail(ti, rots.pop(ti),
                              invB[ti // 4][:, (ti % 4) * 4:(ti % 4) * 4 + 4])

            def _p1_norm_tail(ti, rot, inv):
                nrm = rw.tile([128, 256], BF16, tag="nrm")
                for g in range(4):
                    nc.vector.tensor_scalar_mul(
                        nrm[:, g * 64:(g + 1) * 64],
                        rot[:, g * 64:(g + 1) * 64],
                        inv[:, g:g + 1])
                # normalized q|k into resident [d, t] via DMA XBAR transpose
                b, c0 = ti // 4, (ti % 4) * 128
                nc.sync.dma_start(out=qTs[b][:, c0:c0 + 128],
                                  in_=nrm[:, 0:128], transpose=True)
                nc.sync.dma_start(out=kTs[b][:, c0:c0 + 128],
                                  in_=nrm[:, 128:256], transpose=True)

            def p2(qc, fillers=(), W=512):
                q0 = qc * 512
                visits = []
                for part in range(512 // W):
                    tqa = part * W
                    nkt_p = (q0 + tqa + W) // 128
                    for h in range(H_PER_CORE):
                        visits.append((part, h, nkt_p))
                m_total = sum(v[2] for v in visits)
                emitted = 0
                m = 0
                ypt = yp.tile([128, 512], F32, tag="y")
                for part, h, nkt_p in visits:
                    tqa = part * W
                    hs = h * 64
                    pend = None  # software-pipeline: av trails s by one kt

                    def av(pkt, pat, is_last):
                        # masked cols [0:jcr) of diagonal tiles skipped
                        pjcr = max(pkt * 128 - (q0 + tqa), 0)
                        nc.tensor.matmul(
                            ypt[hs:hs + 64, tqa + pjcr:tqa + W],
                            vNs[pkt // 4][:, pkt % 4, hs:hs + 64],
                            pat[:, pjcr:W],
                            start=(pkt == 0), stop=is_last,
                            skip_group_check=True)

                    for kt in range(nkt_p):
                        while emitted * m_total < m * len(fillers):
                            fillers[emitted]()
                            emitted += 1
                        m += 1
                        kb, kc = kt // 4, (kt % 4) * 128
                        jcr = max(kt * 128 - (q0 + tqa), 0)
                        s_ps = sps.tile([128, W], F32, tag="s")
                        nc.tensor.matmul(
                            s_ps[:, jcr:W],
                            kTs[kb][hs:hs + 64, kc:kc + 128],
                            qTs[qc][hs:hs + 64, tqa + jcr:tqa + W],
                            start=True, stop=True)
                        at = atp.tile([128, W], BF16, tag="at")
                        nc.scalar.activation(
                            at[:, jcr:W], s_ps[:, jcr:W], sig)
                        if kt * 128 >= q0 + tqa:
                            # diagonal 128-block: triangular mask
                            nc.gpsimd.tensor_tensor(
                                at[:, jcr:jcr + 128], at[:, jcr:jcr + 128],
                                tri[:, :], mul)
                        if pend is not None:
                            av(pend[0], pend[1], False)
                        pend = (kt, at)
                    av(pend[0], pend[1], True)
                while emitted < len(fillers):
                    fillers[emitted]()
                    emitted += 1
                nc.vector.tensor_copy(yTs[qc][:, :], ypt[:, :])

            def p3_half(r, half):
                rb, rc = r // 4, (r % 4) * 128
                r0 = r * 128
                o_ps = op.tile([128, 512], F32, tag="o")
                nc.tensor.matmul(
                    o_ps[:, :], yTs[rb][:, rc:rc + 128],
                    wp_b[:, half * 512:(half + 1) * 512],
                    start=True, stop=True)
                ob = osb.tile([128, 512], BF16, tag="ob")
                if r >= 4 * (QC - 1):
                    nc.scalar.activation(ob[:, :], o_ps[:, :],
                                         mybir.ActivationFunctionType.Copy)
                else:
                    nc.vector.tensor_copy(ob[:, :], o_ps[:, :])
                nc.sync.dma_start(
                    out=out_d[r0:r0 + 128, half * 512:(half + 1) * 512],
                    in_=ob[:, :])

            def batch_fillers(b):
                out = [(lambda t=t: p1_compute(t))
                       for t in range(4 * b, 4 * b + 4)]
                out.append(lambda b=b: p1_newton(b))
                out += [(lambda t=t: p1_norm(t))
                        for t in range(4 * b, 4 * b + 4)]
                return out

            for ti in range(12):
                p1_load(ti)
            for b in range(2):
                for fn in batch_fillers(b):
                    fn()
            for qc in range(QC):
                # fillers injected INSIDE p2's kt loop: spreads each p1
                # tile's DVE chain / Pool work thin so queue heads never
                # clog.  Loads first (gpsimd), then computes (batch qc+2,
                # 2-chunk slack) alternating with the previous chunk's
                # projection halves.
                fillers = []
                if qc < QC - 3:
                    fillers += [(lambda t=t: p1_load(t))
                                for t in range(4 * qc + 12, 4 * qc + 16)]
                work = []
                if qc < QC - 2:
                    work += batch_fillers(qc + 2)
                if qc >= 1:
                    p3w = [(lambda r=r, h2=h2: p3_half(r, h2))
                           for r in range(4 * (qc - 1), 4 * qc)
                           for h2 in range(2)]
                else:
                    p3w = []
                ci, pi = 0, 0
                while ci < len(work) or pi < len(p3w):
                    if ci < len(work):
                        fillers.append(work[ci]); ci += 1
                    for _ in range(2):
                        if pi < len(p3w):
                            fillers.append(p3w[pi]); pi += 1
                p2(qc, fillers, W=(128 if qc == 0 else
                                     256 if qc == 1 else 512))
                if qc == 0:
                    nc.sync.dma_start(out=wp_b[:, :], in_=wpT_d[:, :])
            for r in range(4 * (QC - 1), 4 * QC):
                for half in range(2):
                    p3_half(r, half)

    nc.compile()
    return nc


def _in_maps(x, cos, sin, wq, wk, wv, wproj):
    x2d = np.asarray(x, dtype=np.float32).reshape(T, C)
    # tiled transpose: row (128*ti + p) = [x2d[128*ti + j, 128*n + p]
    # for n in 0..7 for j in 0..127] -- contiguous per-partition lines
    xT_bf = np.ascontiguousarray(
        x2d.reshape(TT, 128, C8, 128).transpose(0, 3, 2, 1)
        .reshape(T, C)).astype(BF_NP)
    cos4 = np.ascontiguousarray(
        np.tile(np.asarray(cos, dtype=np.float32), (1, 8))).astype(BF_NP)
    sin4 = np.ascontiguousarray(
        np.tile(np.asarray(sin, dtype=np.float32), (1, 8))).astype(BF_NP)

    in_maps = []
    for c in range(N_CORES):
        sl = slice(c * DSH, (c + 1) * DSH)
        wcat = np.concatenate(
            [wq[sl, :].T, wk[sl, :].T, wv[sl, :].T], axis=1)
        in_maps.append({
            "xT": xT_bf,
            "cos4": cos4,
            "sin4": sin4,
            "wqkv": np.ascontiguousarray(wcat).astype(BF_NP),
            # fold y/(sqrt(T)+1e-6) into the projection weights
            "wpT": np.ascontiguousarray(
                wproj[:, sl].T * np.float32(1.0 / (64.0 + 1e-6))
            ).astype(BF_NP),
        })
    return in_maps


def kernel(x, cos, sin, wq, wk, wv, wproj):
    global _COMPILED
    if _COMPILED is None:
        _COMPILED = _build()
    nc = _COMPILED

    in_maps = _in_maps(x, cos, sin, wq, wk, wv, wproj)
    res = run_bass_kernel_spmd(nc, in_maps, list(range(N_CORES)))
    acc = np.zeros((T, C), dtype=np.float64)
    for c in range(N_CORES):
        acc += np.asarray(res.results[c]["out"], dtype=np.float32)
    return acc.astype(np.float32).reshape(1, T, C)


# revision 26
# speedup vs baseline: 1.1317x; 1.1309x over previous
"""Braid causal self-attention (sigmoid attention + RoPE + QK RMS-norm) on 8
Trainium2 NeuronCores, tensor-parallel over heads (2 heads per core).

Contract: kernel(**inputs) takes the FULL unsharded inputs (numpy) and returns
the FULL output [1, 4096, 1024] float32.

Sharding (host side, per core c):
  - wq/wk/wv rows [128c, 128c+128) == heads 2c, 2c+1, concatenated and
    transposed into one fused [1024, 384] bf16 "wqkv" operand.
  - wproj cols [128c, 128c+128), transposed to [128, 1024] bf16, pre-scaled
    by 1/(sqrt(T)+1e-6).
  - x is pre-transposed on host to xT [1024, 4096] bf16 (so the device needs
    no PE transposes for the QKV contractions).
  - cos/sin are duplicated x4 on host to [4096, 128] bf16 so RoPE group views
    line up without broadcast APs.
  - Each core computes a full-shape partial output y_c @ wproj_c.T; host sums
    the 8 partials.

Device pipeline per core (all bf16 matmuls, fp32 accumulation), fully fused
emission so PE / ACT / DVE / Pool / DMA overlap:
  p1 (per 128-row tile): DMA xT tile, 8 fused-QKV matmuls (N=384) into one
     PSUM bank, RoPE + per-head RMS-norm on DVE, q-hat/k-hat transposed into
     resident qT/kT via DMA XBAR transpose (no PE), v kept natural.
  p2 (per 512-col q chunk, causal tiles only): scoresT = kT.T @ qT (K=64),
     sigmoid on ACT straight out of PSUM, memset/tri-mask on Pool,
     yT += v.T @ attnT accumulated in one PSUM bank for both heads.
  p3 (per 128-row tile): out = yT.T @ wprojT, PSUM -> SBUF -> DRAM f32.
Emission order: p1(0..3), then per qc: p2(qc), p1(next 4 tiles), p3(qc) --
keeps the tensor engine continuously busy (stays at full 2.4 GHz p-state).
"""

import sys

sys.path.insert(0, "/opt/trn_rl_repo")

import numpy as np
import ml_dtypes

import concourse.bass as bass
import concourse.mybir as mybir
from concourse import bacc
from concourse.tile import TileContext
from concourse.bass_utils import run_bass_kernel_spmd
from concourse.masks import make_upper_triangular

T = 4096
C = 1024
N_CORES = 8
D = 64  # head dim
H_PER_CORE = 2  # heads per core
DSH = D * H_PER_CORE  # 128, per-core qkv width
TT = T // 128  # 32 row tiles
C8 = C // 128  # 8 contraction chunks
QC = T // 512  # 8 q chunks
EPS = 1e-6

F32 = mybir.dt.float32
BF16 = mybir.dt.bfloat16
BF_NP = ml_dtypes.bfloat16

_COMPILED = None


def _build():
    nc = bacc.Bacc("TRN2", target_bir_lowering=False, debug=False,
                   num_devices=N_CORES, num_swdge_queues=4)

    xT_d = nc.dram_tensor("xT", [T * 128 // 128, C], BF16, kind="ExternalInput")  # [TT*128, 1024] tiled
    cos4_d = nc.dram_tensor("cos4", [T, 256], BF16, kind="ExternalInput")
    sin4_d = nc.dram_tensor("sin4", [T, 256], BF16, kind="ExternalInput")
    wqkv_d = nc.dram_tensor("wqkv", [C, 384], BF16, kind="ExternalInput")
    wpT_d = nc.dram_tensor("wpT", [DSH, C], BF16, kind="ExternalInput")
    out_d = nc.dram_tensor("out", [T, C], BF16, kind="ExternalOutput")

    mul = mybir.AluOpType.mult
    sig = mybir.ActivationFunctionType.Sigmoid
    sqrtf = mybir.ActivationFunctionType.Sqrt

    with TileContext(nc) as tc:
        with (
            tc.tile_pool(name="const", bufs=1) as constp,
            tc.tile_pool(name="resident", bufs=1) as resp,
            tc.tile_pool(name="xp", bufs=16) as xp,
            tc.tile_pool(name="rw", bufs=2) as rw,
            tc.tile_pool(name="qkvps", bufs=2, space="PSUM") as qkvps,
            tc.tile_pool(name="sps", bufs=3, space="PSUM") as sps,
            tc.tile_pool(name="yp", bufs=2, space="PSUM") as yp,
            tc.tile_pool(name="op", bufs=1, space="PSUM") as op,
            tc.tile_pool(name="atp", bufs=3) as atp,
            tc.tile_pool(name="osb", bufs=2) as osb,
        ):
            # fused qkv weights: wqkv_b[p, c8, j]: contraction chunk c8 rows
            # at partitions p; j = (q 0:128 | k 128:256 | v 256:384).
            # Split into halves so the first QKV matmuls start early.
            wqkv_b = constp.tile([128, C8, 384], BF16)
            wqkv_r = wqkv_d.rearrange("(n p) j -> p n j", p=128)
            nc.gpsimd.dma_start(out=wqkv_b[:, 0:4, :], in_=wqkv_r[:, 0:4, :])
            nc.gpsimd.dma_start(out=wqkv_b[:, 4:8, :], in_=wqkv_r[:, 4:8, :])
            wp_b = constp.tile([128, C], BF16)

            # cos/sin duplicated x4 host-side: [p, tile, 128]; first-half
            # tiles land first so p1(0) RoPE is not blocked.
            cosb = constp.tile([128, TT, 256], BF16)
            sinb = constp.tile([128, TT, 256], BF16)
            cos_r = cos4_d.rearrange("(n p) d -> p n d", p=128)
            sin_r = sin4_d.rearrange("(n p) d -> p n d", p=128)
            nc.gpsimd.dma_start(out=cosb[:, 0:4, :], in_=cos_r[:, 0:4, :])
            nc.gpsimd.dma_start(out=sinb[:, 0:4, :], in_=sin_r[:, 0:4, :])
            nc.gpsimd.dma_start(out=cosb[:, 4:TT, :], in_=cos_r[:, 4:TT, :])
            nc.gpsimd.dma_start(out=sinb[:, 4:TT, :], in_=sin_r[:, 4:TT, :])

            # tri[k, q] = 1 where k <= q (valid causal region of scoresT)
            tri = constp.tile([128, 128], BF16)
            make_upper_triangular(nc, tri, val=1.0, diag=True)

            # resident activations, split per 512-token batch so reads
            # depend only on their batch (dep tracking is per tile object)
            qTs = [resp.tile([128, 512], BF16, tag=f"qT{b}", name=f"qT{b}")
                   for b in range(QC)]  # [d(2 heads), t]
            kTs = [resp.tile([128, 512], BF16, tag=f"kT{b}", name=f"kT{b}")
                   for b in range(QC)]
            vNs = [resp.tile([128, 4, DSH], BF16, tag=f"vN{b}", name=f"vN{b}")
                   for b in range(QC)]  # [t_in_tile, tile_in_batch, d]
            yTs = [resp.tile([128, 512], BF16, tag=f"yT{b}", name=f"yT{b}")
                   for b in range(QC)]  # [d(2 heads), t]


            xts = {}

            def p1_load(ti):
                # host-tiled layout: row (128*ti + p) holds all 8 c-chunks
                # for tile ti, partition p -- one contiguous 2KB line per
                # partition, so SP HWDGE issuance is cheap
                xt = xp.tile([128, C8, 128], BF16, tag="xt")
                nc.sync.dma_start(
                    out=xt[:, :, :],
                    in_=xT_d[ti * 128:(ti + 1) * 128, :]
                    .rearrange("p (n j) -> p n j", n=C8))
                xts[ti] = xt

            def p1_compute(ti):
                r0 = ti * 128
                xt = xts.pop(ti)

                qkv = qkvps.tile([128, 384], F32, tag="qkv")
                for c8 in range(C8):
                    nc.tensor.matmul(qkv[:, :], xt[:, c8, :],
                                     wqkv_b[:, c8, :],
                                     start=(c8 == 0), stop=(c8 == C8 - 1))

                # v: straight cast to resident natural layout
                nc.vector.tensor_copy(vNs[ti // 4][:, ti % 4, :],
                                      qkv[:, 256:384])

                # q|k to bf16 sbuf for cheap (2x-mode) DVE RoPE
                qk = rw.tile([128, 256], BF16, tag="qk")
                nc.vector.tensor_copy(qk[:, :], qkv[:, 0:256])

                # 4 groups g = (q_h0, q_h1, k_h0, k_h1); per group x1 = cols
                # [64g, 64g+32), x2 = [64g+32, 64g+64).  Elementwise p1 work
                # alternates DVE / GpSimd by tile parity to halve the per-
                # batch chain latency (the PSUM copies above stay on DVE).
                ew = nc.vector
                rot = rw.tile([128, 256], BF16, tag="rot")
                qkc = rw.tile([128, 256], BF16, tag="qkc")
                qks = rw.tile([128, 256], BF16, tag="qks")
                # full-width products against x8-duplicated cos/sin tables
                ew.tensor_tensor(qkc[:, :], qk[:, :], cosb[:, ti, :], mul)
                ew.tensor_tensor(qks[:, :], qk[:, :], sinb[:, ti, :], mul)
                gv = "p (g t x) -> p g t x"
                x1c = qkc[:, :].rearrange(gv, g=4, t=2)[:, :, 0:1, :]
                x2c = qkc[:, :].rearrange(gv, g=4, t=2)[:, :, 1:2, :]
                x1s = qks[:, :].rearrange(gv, g=4, t=2)[:, :, 0:1, :]
                x2s = qks[:, :].rearrange(gv, g=4, t=2)[:, :, 1:2, :]
                r1 = rot[:, :].rearrange(gv, g=4, t=2)[:, :, 0:1, :]
                r2 = rot[:, :].rearrange(gv, g=4, t=2)[:, :, 1:2, :]
                # rot1 = x1*c + x2*s ; rot2 = x2*c - x1*s
                ew.tensor_tensor(r1, x1c, x2s, mybir.AluOpType.add)
                ew.tensor_tensor(r2, x2c, x1s, mybir.AluOpType.subtract)

                # per-group sum of squares (rotation preserves the norm)
                ssq = rw.tile([128, 4], F32, tag="ssq")
                sqs = rw.tile([128, 256], BF16, tag="sqs")
                ew.tensor_tensor(sqs[:, :], rot[:, :], rot[:, :], mul)
                nc.vector.reduce_sum(
                    ssq[:, :],
                    sqs[:, :].rearrange("p (g x) -> p g x", g=4),
                    axis=mybir.AxisListType.X)
                # inv = sqrt(1/ssq) via two DVE Newton steps: the ACT
                # engine stays sigmoid-only (one table load for the whole
                # kernel, and p1 never blocks the ACT queue head).
                rec = rw.tile([128, 4], F32, tag="rec")
                nc.vector.reciprocal(rec[:, :], ssq[:, :])
                y0 = rw.tile([128, 4], F32, tag="y0")
                nc.vector.tensor_scalar(y0[:, :], rec[:, :], 2.3667, 0.0959,
                                        mul, mybir.AluOpType.add)
                r1n = rw.tile([128, 4], F32, tag="r1n")
                nc.vector.reciprocal(r1n[:, :], y0[:, :])
                t1n = rw.tile([128, 4], F32, tag="t1n")
                nc.vector.tensor_tensor(t1n[:, :], rec[:, :], r1n[:, :], mul)
                y1u = rw.tile([128, 4], F32, tag="y1u")  # = 2*y1
                nc.vector.tensor_tensor(y1u[:, :], y0[:, :], t1n[:, :],
                                        mybir.AluOpType.add)
                r2n = rw.tile([128, 4], F32, tag="r2n")  # = 1/(2*y1)
                nc.vector.reciprocal(r2n[:, :], y1u[:, :])
                t2n = rw.tile([128, 4], F32, tag="t2n")  # = a/(2*y1)
                nc.vector.tensor_tensor(t2n[:, :], rec[:, :], r2n[:, :], mul)
                inv = rw.tile([128, 4], F32, tag="inv")
                nc.vector.tensor_scalar(inv[:, :], y1u[:, :], 0.25, None,
                                        mul)
                nc.vector.tensor_tensor(inv[:, :], inv[:, :], t2n[:, :],
                                        mybir.AluOpType.add)
                nc.vector.tensor_scalar_mul(inv[:, 2:4], inv[:, 2:4], 8.0)
                _p1_norm_tail(ti, rot, inv)

            def _p1_norm_tail(ti, rot, inv):
                nrm = rw.tile([128, 256], BF16, tag="nrm")
                for g in range(4):
                    nc.vector.tensor_scalar_mul(
                        nrm[:, g * 64:(g + 1) * 64],
                        rot[:, g * 64:(g + 1) * 64],
                        inv[:, g:g + 1])
                # normalized q|k into resident [d, t] via DMA XBAR transpose
                b, c0 = ti // 4, (ti % 4) * 128
                nc.sync.dma_start(out=qTs[b][:, c0:c0 + 128],
                                  in_=nrm[:, 0:128], transpose=True)
                nc.sync.dma_start(out=kTs[b][:, c0:c0 + 128],
                                  in_=nrm[:, 128:256], transpose=True)

            def p2(qc, fillers=(), W=512):
                q0 = qc * 512
                visits = []
                for part in range(512 // W):
                    tqa = part * W
                    nkt_p = (q0 + tqa + W) // 128
                    for h in range(H_PER_CORE):
                        visits.append((part, h, nkt_p))
                m_total = sum(v[2] for v in visits)
                emitted = 0
                m = 0
                ypt = yp.tile([128, 512], F32, tag="y")
                for part, h, nkt_p in visits:
                    tqa = part * W
                    hs = h * 64
                    pend = None  # software-pipeline: av trails s by one kt

                    def av(pkt, pat, is_last):
                        # masked cols [0:jcr) of diagonal tiles skipped
                        pjcr = max(pkt * 128 - (q0 + tqa), 0)
                        nc.tensor.matmul(
                            ypt[hs:hs + 64, tqa + pjcr:tqa + W],
                            vNs[pkt // 4][:, pkt % 4, hs:hs + 64],
                            pat[:, pjcr:W],
                            start=(pkt == 0), stop=is_last,
                            skip_group_check=True)

                    for kt in range(nkt_p):
                        while emitted * m_total < m * len(fillers):
                            fillers[emitted]()
                            emitted += 1
                        m += 1
                        kb, kc = kt // 4, (kt % 4) * 128
                        jcr = max(kt * 128 - (q0 + tqa), 0)
                        s_ps = sps.tile([128, W], F32, tag="s")
                        nc.tensor.matmul(
                            s_ps[:, jcr:W],
                            kTs[kb][hs:hs + 64, kc:kc + 128],
                            qTs[qc][hs:hs + 64, tqa + jcr:tqa + W],
                            start=True, stop=True)
                        at = atp.tile([128, W], BF16, tag="at")
                        nc.scalar.activation(
                            at[:, jcr:W], s_ps[:, jcr:W], sig)
                        if kt * 128 >= q0 + tqa:
                            # diagonal 128-block: triangular mask
                            nc.gpsimd.tensor_tensor(
                                at[:, jcr:jcr + 128], at[:, jcr:jcr + 128],
                                tri[:, :], mul)
                        if pend is not None:
                            av(pend[0], pend[1], False)
                        pend = (kt, at)
                    av(pend[0], pend[1], True)
                while emitted < len(fillers):
                    fillers[emitted]()
                    emitted += 1
                nc.vector.tensor_copy(yTs[qc][:, :], ypt[:, :])

            def p3_half(r, half):
                rb, rc = r // 4, (r % 4) * 128
                r0 = r * 128
                o_ps = op.tile([128, 512], F32, tag="o")
                nc.tensor.matmul(
                    o_ps[:, :], yTs[rb][:, rc:rc + 128],
                    wp_b[:, half * 512:(half + 1) * 512],
                    start=True, stop=True)
                ob = osb.tile([128, 512], BF16, tag="ob")
                if r >= 4 * (QC - 1):
                    nc.scalar.activation(ob[:, :], o_ps[:, :],
                                         mybir.ActivationFunctionType.Copy)
                else:
                    nc.vector.tensor_copy(ob[:, :], o_ps[:, :])
                nc.sync.dma_start(
                    out=out_d[r0:r0 + 128, half * 512:(half + 1) * 512],
                    in_=ob[:, :])

            for ti in range(12):
                p1_load(ti)
            for ti in range(8):
                p1_compute(ti)
            for qc in range(QC):
                # fillers injected INSIDE p2's kt loop: spreads each p1
                # tile's DVE chain / Pool work thin so queue heads never
                # clog.  Loads first (gpsimd), then computes (batch qc+2,
                # 2-chunk slack) alternating with the previous chunk's
                # projection halves.
                fillers = []
                if qc < QC - 3:
                    fillers += [(lambda t=t: p1_load(t))
                                for t in range(4 * qc + 12, 4 * qc + 16)]
                work = []
                if qc < QC - 2:
                    work += [(lambda t=t: p1_compute(t))
                             for t in range(4 * qc + 8, 4 * qc + 12)]
                if qc >= 1:
                    p3w = [(lambda r=r, h2=h2: p3_half(r, h2))
                           for r in range(4 * (qc - 1), 4 * qc)
                           for h2 in range(2)]
                else:
                    p3w = []
                ci, pi = 0, 0
                while ci < len(work) or pi < len(p3w):
                    if ci < len(work):
                        fillers.append(work[ci]); ci += 1
                    for _ in range(2):
                        if pi < len(p3w):
                            fillers.append(p3w[pi]); pi += 1
                p2(qc, fillers, W=(128 if qc == 0 else
                                     256 if qc == 1 else 512))
                if qc == 0:
                    nc.sync.dma_start(out=wp_b[:, :], in_=wpT_d[:, :])
            for r in range(4 * (QC - 1), 4 * QC):
                for half in range(2):
                    p3_half(r, half)

    nc.compile()
    return nc


def _in_maps(x, cos, sin, wq, wk, wv, wproj):
    x2d = np.asarray(x, dtype=np.float32).reshape(T, C)
    # tiled transpose: row (128*ti + p) = [x2d[128*ti + j, 128*n + p]
    # for n in 0..7 for j in 0..127] -- contiguous per-partition lines
    xT_bf = np.ascontiguousarray(
        x2d.reshape(TT, 128, C8, 128).transpose(0, 3, 2, 1)
        .reshape(T, C)).astype(BF_NP)
    cos4 = np.ascontiguousarray(
        np.tile(np.asarray(cos, dtype=np.float32), (1, 8))).astype(BF_NP)
    sin4 = np.ascontiguousarray(
        np.tile(np.asarray(sin, dtype=np.float32), (1, 8))).astype(BF_NP)

    in_maps = []
    for c in range(N_CORES):
        sl = slice(c * DSH, (c + 1) * DSH)
        wcat = np.concatenate(
            [wq[sl, :].T, wk[sl, :].T, wv[sl, :].T], axis=1)
        in_maps.append({
            "xT": xT_bf,
            "cos4": cos4,
            "sin4": sin4,
            "wqkv": np.ascontiguousarray(wcat).astype(BF_NP),
            # fold y/(sqrt(T)+1e-6) into the projection weights
            "wpT": np.ascontiguousarray(
                wproj[:, sl].T * np.float32(1.0 / (64.0 + 1e-6))
            ).astype(BF_NP),
        })
    return in_maps


def kernel(x, cos, sin, wq, wk, wv, wproj):
    global _COMPILED
    if _COMPILED is None:
        _COMPILED = _build()
    nc = _COMPILED

    in_maps = _in_maps(x, cos, sin, wq, wk, wv, wproj)
    res = run_bass_kernel_spmd(nc, in_maps, list(range(N_CORES)))
    acc = np.zeros((T, C), dtype=np.float64)
    for c in range(N_CORES):
        acc += np.asarray(res.results[c]["out"], dtype=np.float32)
    return acc.astype(np.float32).reshape(1, T, C)


# revision 28
# speedup vs baseline: 1.1999x; 1.0603x over previous
"""Braid causal self-attention (sigmoid attention + RoPE + QK RMS-norm) on 8
Trainium2 NeuronCores, tensor-parallel over heads (2 heads per core).

Contract: kernel(**inputs) takes the FULL unsharded inputs (numpy) and returns
the FULL output [1, 4096, 1024] float32.

Sharding (host side, per core c):
  - wq/wk/wv rows [128c, 128c+128) == heads 2c, 2c+1, concatenated and
    transposed into one fused [1024, 384] bf16 "wqkv" operand.
  - wproj cols [128c, 128c+128), transposed to [128, 1024] bf16, pre-scaled
    by 1/(sqrt(T)+1e-6).
  - x is pre-transposed AND tile-blocked on host to bf16 (one contiguous 2KB
    line per SBUF partition per tile -> cheap HWDGE loads, no PE transposes).
  - cos/sin are duplicated x8 on host to [4096, 256] bf16 so the RoPE
    products are two full-width DVE ops without broadcast APs.
  - Each core computes a full-shape bf16 partial output y_c @ wproj_c.T;
    host sums the 8 partials in float64.

Device pipeline per core (all bf16 matmuls, fp32 accumulation), fully fused
emission so PE / ACT / DVE / Pool / DMA overlap:
  p1 (per 128-row tile): DMA x tile, 8 fused-QKV matmuls (N=384) into one
     PSUM bank, RoPE + per-head RMS-norm on DVE (rsqrt via chord-seeded
     Newton so the ACT engine stays sigmoid-only: one activation-table load
     for the whole kernel), q-hat/k-hat into per-batch resident tiles via
     DMA XBAR transpose (no PE), v kept natural.
  p2 (per q chunk; 128/256-wide windows for the first two chunks so the
     warmup depends on fewer tiles): scoresT = kT.T @ qT (K=64, diagonal
     tiles narrowed to live columns), sigmoid on ACT straight out of PSUM,
     tri-mask on Pool, yT += v.T @ attnT accumulated in one PSUM bank for
     both heads, with the av matmul software-pipelined one kt behind the
     score matmul.
  p3 (per 128-row tile, one chunk delayed): out = yT.T @ wprojT,
     PSUM -> SBUF -> DRAM bf16 partials.
p1 loads/computes and p3 halves are injected as fillers inside p2's kt loop
(2-chunk lookahead) so no engine queue ever clogs at its head.
"""

import sys

sys.path.insert(0, "/opt/trn_rl_repo")

import numpy as np
import ml_dtypes

import concourse.bass as bass
import concourse.mybir as mybir
from concourse import bacc
from concourse.tile import TileContext
from concourse.bass_utils import run_bass_kernel_spmd
from concourse.masks import make_upper_triangular

T = 4096
C = 1024
N_CORES = 8
D = 64  # head dim
H_PER_CORE = 2  # heads per core
DSH = D * H_PER_CORE  # 128, per-core qkv width
TT = T // 128  # 32 row tiles
C8 = C // 128  # 8 contraction chunks
QC = T // 512  # 8 q chunks
EPS = 1e-6

F32 = mybir.dt.float32
BF16 = mybir.dt.bfloat16
BF_NP = ml_dtypes.bfloat16

_COMPILED = None


def _build():
    nc = bacc.Bacc("TRN2", target_bir_lowering=False, debug=False,
                   num_devices=N_CORES, num_swdge_queues=4)

    xT_d = nc.dram_tensor("xT", [T * 128 // 128, C], BF16, kind="ExternalInput")  # [TT*128, 1024] tiled
    cos4_d = nc.dram_tensor("cos4", [T, 256], BF16, kind="ExternalInput")
    sin4_d = nc.dram_tensor("sin4", [T, 256], BF16, kind="ExternalInput")
    wqkv_d = nc.dram_tensor("wqkv", [C, 384], BF16, kind="ExternalInput")
    wpT_d = nc.dram_tensor("wpT", [DSH, C], BF16, kind="ExternalInput")
    out_d = nc.dram_tensor("out", [T, C], BF16, kind="ExternalOutput")

    mul = mybir.AluOpType.mult
    sig = mybir.ActivationFunctionType.Sigmoid
    sqrtf = mybir.ActivationFunctionType.Sqrt

    with TileContext(nc) as tc:
        with (
            tc.tile_pool(name="const", bufs=1) as constp,
            tc.tile_pool(name="resident", bufs=1) as resp,
            tc.tile_pool(name="xp", bufs=16) as xp,
            tc.tile_pool(name="rw", bufs=2) as rw,
            tc.tile_pool(name="qkvps", bufs=2, space="PSUM") as qkvps,
            tc.tile_pool(name="sps", bufs=3, space="PSUM") as sps,
            tc.tile_pool(name="yp", bufs=2, space="PSUM") as yp,
            tc.tile_pool(name="op", bufs=1, space="PSUM") as op,
            tc.tile_pool(name="atp", bufs=3) as atp,
            tc.tile_pool(name="osb", bufs=2) as osb,
        ):
            # fused qkv weights: wqkv_b[p, c8, j]: contraction chunk c8 rows
            # at partitions p; j = (q 0:128 | k 128:256 | v 256:384).
            # Split into halves so the first QKV matmuls start early.
            wqkv_b = constp.tile([128, C8, 384], BF16)
            wqkv_r = wqkv_d.rearrange("(n p) j -> p n j", p=128)
            nc.gpsimd.dma_start(out=wqkv_b[:, 0:4, :], in_=wqkv_r[:, 0:4, :])
            nc.gpsimd.dma_start(out=wqkv_b[:, 4:8, :], in_=wqkv_r[:, 4:8, :])
            wp_b = constp.tile([128, C], BF16)

            # cos/sin duplicated x4 host-side: [p, tile, 128]; first-half
            # tiles land first so p1(0) RoPE is not blocked.
            cosb = constp.tile([128, TT, 256], BF16)
            sinb = constp.tile([128, TT, 256], BF16)
            cos_r = cos4_d.rearrange("(n p) d -> p n d", p=128)
            sin_r = sin4_d.rearrange("(n p) d -> p n d", p=128)
            nc.gpsimd.dma_start(out=cosb[:, 0:4, :], in_=cos_r[:, 0:4, :])
            nc.gpsimd.dma_start(out=sinb[:, 0:4, :], in_=sin_r[:, 0:4, :])
            nc.gpsimd.dma_start(out=cosb[:, 4:TT, :], in_=cos_r[:, 4:TT, :])
            nc.gpsimd.dma_start(out=sinb[:, 4:TT, :], in_=sin_r[:, 4:TT, :])

            # tri[k, q] = 1 where k <= q (valid causal region of scoresT)
            tri = constp.tile([128, 128], BF16)
            make_upper_triangular(nc, tri, val=1.0, diag=True)

            # resident activations, split per 512-token batch so reads
            # depend only on their batch (dep tracking is per tile object)
            qTs = [resp.tile([128, 512], BF16, tag=f"qT{b}", name=f"qT{b}")
                   for b in range(QC)]  # [d(2 heads), t]
            # per-tile q/k for the first 8 tiles: warmup chunks then depend
            # on exactly the producer tile instead of a whole 4-tile batch
            qTt = [resp.tile([128, 128], BF16, tag=f"qt{i}", name=f"qt{i}")
                   for i in range(8)]
            kTt = [resp.tile([128, 128], BF16, tag=f"kt{i}", name=f"kt{i}")
                   for i in range(8)]
            kTs = [resp.tile([128, 512], BF16, tag=f"kT{b}", name=f"kT{b}")
                   for b in range(QC)]
            vNs = [resp.tile([128, 4, DSH], BF16, tag=f"vN{b}", name=f"vN{b}")
                   for b in range(QC)]  # [t_in_tile, tile_in_batch, d]
            yTs = [resp.tile([128, 512], BF16, tag=f"yT{b}", name=f"yT{b}")
                   for b in range(QC)]  # [d(2 heads), t]


            xts = {}

            def p1_load(ti):
                # host-tiled layout: row (128*ti + p) holds all 8 c-chunks
                # for tile ti, partition p -- one contiguous 2KB line per
                # partition, so SP HWDGE issuance is cheap
                xt = xp.tile([128, C8, 128], BF16, tag="xt")
                nc.sync.dma_start(
                    out=xt[:, :, :],
                    in_=xT_d[ti * 128:(ti + 1) * 128, :]
                    .rearrange("p (n j) -> p n j", n=C8))
                xts[ti] = xt

            def p1_compute(ti):
                r0 = ti * 128
                xt = xts.pop(ti)

                qkv = qkvps.tile([128, 384], F32, tag="qkv")
                for c8 in range(C8):
                    nc.tensor.matmul(qkv[:, :], xt[:, c8, :],
                                     wqkv_b[:, c8, :],
                                     start=(c8 == 0), stop=(c8 == C8 - 1))

                # v: straight cast to resident natural layout
                nc.vector.tensor_copy(vNs[ti // 4][:, ti % 4, :],
                                      qkv[:, 256:384])

                # q|k to bf16 sbuf for cheap (2x-mode) DVE RoPE
                qk = rw.tile([128, 256], BF16, tag="qk")
                nc.vector.tensor_copy(qk[:, :], qkv[:, 0:256])

                # 4 groups g = (q_h0, q_h1, k_h0, k_h1); per group x1 = cols
                # [64g, 64g+32), x2 = [64g+32, 64g+64).  Elementwise p1 work
                # alternates DVE / GpSimd by tile parity to halve the per-
                # batch chain latency (the PSUM copies above stay on DVE).
                ew = nc.vector
                rot = rw.tile([128, 256], BF16, tag="rot")
                qkc = rw.tile([128, 256], BF16, tag="qkc")
                qks = rw.tile([128, 256], BF16, tag="qks")
                # full-width products against x8-duplicated cos/sin tables
                ew.tensor_tensor(qkc[:, :], qk[:, :], cosb[:, ti, :], mul)
                ew.tensor_tensor(qks[:, :], qk[:, :], sinb[:, ti, :], mul)
                gv = "p (g t x) -> p g t x"
                x1c = qkc[:, :].rearrange(gv, g=4, t=2)[:, :, 0:1, :]
                x2c = qkc[:, :].rearrange(gv, g=4, t=2)[:, :, 1:2, :]
                x1s = qks[:, :].rearrange(gv, g=4, t=2)[:, :, 0:1, :]
                x2s = qks[:, :].rearrange(gv, g=4, t=2)[:, :, 1:2, :]
                r1 = rot[:, :].rearrange(gv, g=4, t=2)[:, :, 0:1, :]
                r2 = rot[:, :].rearrange(gv, g=4, t=2)[:, :, 1:2, :]
                # rot1 = x1*c + x2*s ; rot2 = x2*c - x1*s
                ew.tensor_tensor(r1, x1c, x2s, mybir.AluOpType.add)
                ew.tensor_tensor(r2, x2c, x1s, mybir.AluOpType.subtract)

                # per-group sum of squares (rotation preserves the norm)
                ssq = rw.tile([128, 4], F32, tag="ssq")
                sqs = rw.tile([128, 256], BF16, tag="sqs")
                ew.tensor_tensor(sqs[:, :], rot[:, :], rot[:, :], mul)
                nc.vector.reduce_sum(
                    ssq[:, :],
                    sqs[:, :].rearrange("p (g x) -> p g x", g=4),
                    axis=mybir.AxisListType.X)
                # inv = sqrt(1/ssq) via two DVE Newton steps: the ACT
                # engine stays sigmoid-only (one table load for the whole
                # kernel, and p1 never blocks the ACT queue head).
                rec = rw.tile([128, 4], F32, tag="rec")
                nc.vector.reciprocal(rec[:, :], ssq[:, :])
                y0 = rw.tile([128, 4], F32, tag="y0")
                nc.vector.tensor_scalar(y0[:, :], rec[:, :], 2.3667, 0.0959,
                                        mul, mybir.AluOpType.add)
                r1n = rw.tile([128, 4], F32, tag="r1n")
                nc.vector.reciprocal(r1n[:, :], y0[:, :])
                t1n = rw.tile([128, 4], F32, tag="t1n")
                nc.vector.tensor_tensor(t1n[:, :], rec[:, :], r1n[:, :], mul)
                y1u = rw.tile([128, 4], F32, tag="y1u")  # = 2*y1
                nc.vector.tensor_tensor(y1u[:, :], y0[:, :], t1n[:, :],
                                        mybir.AluOpType.add)
                r2n = rw.tile([128, 4], F32, tag="r2n")  # = 1/(2*y1)
                nc.vector.reciprocal(r2n[:, :], y1u[:, :])
                t2n = rw.tile([128, 4], F32, tag="t2n")  # = a/(2*y1)
                nc.vector.tensor_tensor(t2n[:, :], rec[:, :], r2n[:, :], mul)
                inv = rw.tile([128, 4], F32, tag="inv")
                nc.vector.tensor_scalar(inv[:, :], y1u[:, :], 0.25, None,
                                        mul)
                nc.vector.tensor_tensor(inv[:, :], inv[:, :], t2n[:, :],
                                        mybir.AluOpType.add)
                nc.vector.tensor_scalar_mul(inv[:, 2:4], inv[:, 2:4], 8.0)
                _p1_norm_tail(ti, rot, inv)

            def _p1_norm_tail(ti, rot, inv):
                nrm = rw.tile([128, 256], BF16, tag="nrm")
                for g in range(4):
                    nc.vector.tensor_scalar_mul(
                        nrm[:, g * 64:(g + 1) * 64],
                        rot[:, g * 64:(g + 1) * 64],
                        inv[:, g:g + 1])
                # normalized q|k into resident [d, t] via DMA XBAR transpose
                b, c0 = ti // 4, (ti % 4) * 128
                if ti < 8:
                    nc.sync.dma_start(out=qTt[ti][:, :],
                                      in_=nrm[:, 0:128], transpose=True)
                    nc.sync.dma_start(out=kTt[ti][:, :],
                                      in_=nrm[:, 128:256], transpose=True)
                else:
                    nc.sync.dma_start(out=qTs[b][:, c0:c0 + 128],
                                      in_=nrm[:, 0:128], transpose=True)
                    nc.sync.dma_start(out=kTs[b][:, c0:c0 + 128],
                                      in_=nrm[:, 128:256], transpose=True)

            def p2(qc, fillers=(), W=512):
                q0 = qc * 512
                visits = []
                for part in range(512 // W):
                    tqa = part * W
                    nkt_p = (q0 + tqa + W) // 128
                    for h in range(H_PER_CORE):
                        visits.append((part, h, nkt_p))
                m_total = sum(v[2] for v in visits)
                emitted = 0
                m = 0
                ypt = yp.tile([128, 512], F32, tag="y")
                for part, h, nkt_p in visits:
                    tqa = part * W
                    hs = h * 64
                    pend = None  # software-pipeline: av trails s by one kt

                    def av(pkt, pat, is_last):
                        # masked cols [0:jcr) of diagonal tiles skipped
                        pjcr = max(pkt * 128 - (q0 + tqa), 0)
                        nc.tensor.matmul(
                            ypt[hs:hs + 64, tqa + pjcr:tqa + W],
                            vNs[pkt // 4][:, pkt % 4, hs:hs + 64],
                            pat[:, pjcr:W],
                            start=(pkt == 0), stop=is_last,
                            skip_group_check=True)

                    for kt in range(nkt_p):
                        while emitted * m_total < m * len(fillers):
                            fillers[emitted]()
                            emitted += 1
                        m += 1
                        kb, kc = kt // 4, (kt % 4) * 128
                        jcr = max(kt * 128 - (q0 + tqa), 0)
                        if kt < 8:
                            k_lhs = kTt[kt][hs:hs + 64, :]
                        else:
                            k_lhs = kTs[kb][hs:hs + 64, kc:kc + 128]
                        if qc < 2:
                            q_rhs = qTt[4 * qc + part][hs:hs + 64, jcr:W]
                        else:
                            q_rhs = qTs[qc][hs:hs + 64, tqa + jcr:tqa + W]
                        s_ps = sps.tile([128, W], F32, tag="s")
                        nc.tensor.matmul(
                            s_ps[:, jcr:W],
                            k_lhs, q_rhs,
                            start=True, stop=True)
                        at = atp.tile([128, W], BF16, tag="at")
                        nc.scalar.activation(
                            at[:, jcr:W], s_ps[:, jcr:W], sig)
                        if kt * 128 >= q0 + tqa:
                            # diagonal 128-block: triangular mask
                            nc.gpsimd.tensor_tensor(
                                at[:, jcr:jcr + 128], at[:, jcr:jcr + 128],
                                tri[:, :], mul)
                        if pend is not None:
                            av(pend[0], pend[1], False)
                        pend = (kt, at)
                    av(pend[0], pend[1], True)
                while emitted < len(fillers):
                    fillers[emitted]()
                    emitted += 1
                nc.vector.tensor_copy(yTs[qc][:, :], ypt[:, :])

            def p3_half(r, half):
                rb, rc = r // 4, (r % 4) * 128
                r0 = r * 128
                o_ps = op.tile([128, 512], F32, tag="o")
                nc.tensor.matmul(
                    o_ps[:, :], yTs[rb][:, rc:rc + 128],
                    wp_b[:, half * 512:(half + 1) * 512],
                    start=True, stop=True)
                ob = osb.tile([128, 512], BF16, tag="ob")
                if r >= 4 * (QC - 1):
                    nc.scalar.activation(ob[:, :], o_ps[:, :],
                                         mybir.ActivationFunctionType.Copy)
                else:
                    nc.vector.tensor_copy(ob[:, :], o_ps[:, :])
                nc.sync.dma_start(
                    out=out_d[r0:r0 + 128, half * 512:(half + 1) * 512],
                    in_=ob[:, :])

            for ti in range(12):
                p1_load(ti)
            for ti in range(8):
                p1_compute(ti)
            for qc in range(QC):
                # fillers injected INSIDE p2's kt loop: spreads each p1
                # tile's DVE chain / Pool work thin so queue heads never
                # clog.  Loads first (gpsimd), then computes (batch qc+2,
                # 2-chunk slack) alternating with the previous chunk's
                # projection halves.
                fillers = []
                if qc < QC - 3:
                    fillers += [(lambda t=t: p1_load(t))
                                for t in range(4 * qc + 12, 4 * qc + 16)]
                work = []
                if qc < QC - 2:
                    work += [(lambda t=t: p1_compute(t))
                             for t in range(4 * qc + 8, 4 * qc + 12)]
                if qc >= 1:
                    p3w = [(lambda r=r, h2=h2: p3_half(r, h2))
                           for r in range(4 * (qc - 1), 4 * qc)
                           for h2 in range(2)]
                else:
                    p3w = []
                ci, pi = 0, 0
                while ci < len(work) or pi < len(p3w):
                    if ci < len(work):
                        fillers.append(work[ci]); ci += 1
                    for _ in range(2):
                        if pi < len(p3w):
                            fillers.append(p3w[pi]); pi += 1
                p2(qc, fillers, W=(128 if qc < 2 else 512))
                if qc == 0:
                    nc.sync.dma_start(out=wp_b[:, :], in_=wpT_d[:, :])
            for r in range(4 * (QC - 1), 4 * QC):
                for half in range(2):
                    p3_half(r, half)

    nc.compile()
    return nc


def _in_maps(x, cos, sin, wq, wk, wv, wproj):
    x2d = np.asarray(x, dtype=np.float32).reshape(T, C)
    # tiled transpose: row (128*ti + p) = [x2d[128*ti + j, 128*n + p]
    # for n in 0..7 for j in 0..127] -- contiguous per-partition lines
    xT_bf = np.ascontiguousarray(
        x2d.reshape(TT, 128, C8, 128).transpose(0, 3, 2, 1)
        .reshape(T, C)).astype(BF_NP)
    cos4 = np.ascontiguousarray(
        np.tile(np.asarray(cos, dtype=np.float32), (1, 8))).astype(BF_NP)
    sin4 = np.ascontiguousarray(
        np.tile(np.asarray(sin, dtype=np.float32), (1, 8))).astype(BF_NP)

    in_maps = []
    for c in range(N_CORES):
        sl = slice(c * DSH, (c + 1) * DSH)
        wcat = np.concatenate(
            [wq[sl, :].T, wk[sl, :].T, wv[sl, :].T], axis=1)
        in_maps.append({
            "xT": xT_bf,
            "cos4": cos4,
            "sin4": sin4,
            "wqkv": np.ascontiguousarray(wcat).astype(BF_NP),
            # fold y/(sqrt(T)+1e-6) into the projection weights
            "wpT": np.ascontiguousarray(
                wproj[:, sl].T * np.float32(1.0 / (64.0 + 1e-6))
            ).astype(BF_NP),
        })
    return in_maps


def kernel(x, cos, sin, wq, wk, wv, wproj):
    global _COMPILED
    if _COMPILED is None:
        _COMPILED = _build()
    nc = _COMPILED

    in_maps = _in_maps(x, cos, sin, wq, wk, wv, wproj)
    res = run_bass_kernel_spmd(nc, in_maps, list(range(N_CORES)))
    acc = np.zeros((T, C), dtype=np.float64)
    for c in range(N_CORES):
        acc += np.asarray(res.results[c]["out"], dtype=np.float32)
    return acc.astype(np.float32).reshape(1, T, C)
